# revision 6
# baseline (speedup 1.0000x reference)
"""ConvGRU Trainium2 kernel, v3 (v2 + flat-contiguous DMA).

Same compute structure as v2 (14 K-packed matmuls/chunk via shift-baked
A/B/C rhs buffers; bf16 GRU chain; bf16 h state inside the buffers), but all
hot DMAs are single-run-per-partition flat copies:

- rhs buffers are [128, PR*PC] with PC=70. A shift by (dy, dx) is a flat
  element offset of -(70*dy + dx), so shifted replicas/loads are contiguous
  flat copies whose row-wrap elements land in always-zero pad columns.
- x arrives from DRAM pre-padded per core as xp[T, CIN, PR*PC] (canonical
  position s=1+r, u=2+c); the 5 per-step loads are flat shifted copies.
- h replicas (dx=-1 into A lower, dx=+1 into B lower) are flat +-1 copies
  of the canonical h@dx=0 chunk slice.
- output is one flat per-step DMA of the owned 32 rows (with pad cols),
  descrambled and upcast to f32 on the host.
"""

import os
import sys

sys.path.insert(0, "/opt/trn_rl_repo")

import numpy as np

T, CIN, HID, H, W = 16, 32, 64, 64, 64
B = 4
NCORES = 8
OWN = 32           # owned H rows per core
XR = 48            # x slice rows fed to each core
PR = 50            # padded rows in rhs buffers
PC = 70            # padded cols (even: keeps DVE rows 4B-aligned)
CHUNK = 8          # output rows per chunk (8*64 = 512 = one PSUM bank)
FLAT = PR * PC
OUTSZ = OWN * PC   # owned-rows flat slice per step (rows 1..32)

_CACHE = {}
KERNEL_STATS = {}


def _n_rows(t):
    return OWN + (T - t)


def _build():
    import concourse.bacc as bacc
    import concourse.mybir as mybir
    from concourse import tile

    dt = mybir.dt
    AF = mybir.ActivationFunctionType

    nc = bacc.Bacc("TRN2", target_bir_lowering=False, debug=False,
                   num_devices=NCORES)
    xin = nc.dram_tensor("xin", [T, CIN, FLAT], dt.bfloat16,
                         kind="ExternalInput")
    w1 = nc.dram_tensor("w1", [128, 7 * 128], dt.bfloat16,
                        kind="ExternalInput")
    w2 = nc.dram_tensor("w2", [128, 7 * 128], dt.bfloat16,
                        kind="ExternalInput")
    out = nc.dram_tensor("out", [T, HID, OUTSZ], dt.bfloat16,
                         kind="ExternalOutput")

    # x data occupies flat [70, 3430) (rows 1..48, canonical). A target with
    # baked shift (dy, dx) holds it at flat offset d = -(70*dy + dx).
    xspan = (PC, (1 + XR) * PC)

    with tile.TileContext(nc) as tc:
        with tc.tile_pool(name="const", bufs=1) as const, \
             tc.tile_pool(name="state", bufs=1) as state, \
             tc.tile_pool(name="work", bufs=3) as work, \
             tc.tile_pool(name="psum", bufs=3, space="PSUM") as psum, \
             tc.tile_pool(name="psum3", bufs=2, space="PSUM") as psum3:

            w1s = const.tile([128, 7 * 128], dt.bfloat16, tag="w1s")
            w2s = const.tile([128, 7 * 128], dt.bfloat16, tag="w2s")
            # per-block loads so MM 0 only waits on its own 128 columns
            for mi in range(7):
                sl = slice(mi * 128, (mi + 1) * 128)
                nc.sync.dma_start(w1s[:, sl], w1[:, sl])
                nc.sync.dma_start(w2s[:, sl], w2[:, sl])

            # zero-fill spread across engines so startup is not serialized
            # on one engine (DVE memset on [128, 3500] bf16 runs ~1 us)
            bufs, bufs3 = [], []
            zeng = [lambda ap: nc.vector.memset(ap, 0.0),
                    lambda ap: nc.scalar.memzero(ap),
                    lambda ap: nc.gpsimd.memset(ap, 0.0)]
            for s in range(3):
                trio, trio3 = [], []
                for k, nm in enumerate(("B", "C", "A")):
                    tl = state.tile([128, FLAT], dt.bfloat16,
                                    tag=f"{nm}{s}", name=f"{nm}{s}")
                    if s == 0:
                        # set 0 gates the first matmuls: split halves
                        # across engines so each clear is ~1.5 us
                        zeng[k](tl[0:64, :])
                        zeng[(k + 1) % 3](tl[64:128, :])
                    else:
                        zeng[k](tl[:])
                    trio.append(tl)
                    trio3.append(tl.rearrange("p (r c) -> p r c", c=PC))
                # emitted B, C, A; store as A, B, C
                bufs.append([trio[2], trio[0], trio[1]])
                bufs3.append([trio3[2], trio3[0], trio3[1]])

            def load_x(t, ds, split=None):
                # (partition block, dy, dx) -> flat shifted copy of xin[t].
                # split: emit rows < split first so early chunks can start.
                _, Bb, Cb = bufs[ds]
                lo, hi = xspan
                tgts = ((Bb, 64, 0, -1), (Bb, 96, 0, 0),
                        (Cb, 0, -1, 1), (Cb, 32, 0, 1), (Cb, 64, 1, 1))
                pieces = [(lo, hi)] if split is None else \
                    [(lo, lo + split * PC), (lo + split * PC, hi)]
                for plo, phi in pieces:
                    for tl, p0, dy, dx in tgts:
                        d = -(PC * dy + dx)
                        s0, s1 = max(0, plo + d), min(FLAT, phi + d)
                        nc.sync.dma_start(tl[p0:p0 + 32, s0:s1],
                                          xin[t, :, s0 - d:s1 - d])

            load_x(0, 0, split=12)

            for t in range(1, T + 1):
                nt = _n_rows(t)
                cur = bufs3[(t - 1) % 3]
                An_f, Bn_f, _ = bufs[t % 3]
                An = bufs3[t % 3][0]

                if t < T:
                    load_x(t, t % 3)

                nchunks = (nt + CHUNK - 1) // CHUNK
                # top-halo chunk second: its (short) chain+replicas then
                # finish well before the step's MM stream does, so the next
                # step's first matmuls never wait on a trailing chain.
                order = list(range(nchunks))
                if nchunks > 2:
                    order = [0, nchunks - 1] + order[1:-1]
                for ci in order:
                    r0 = ci * CHUNK
                    nr = min(CHUNK, nt - r0)
                    N = nr * W
                    p1 = psum.tile([128, CHUNK * W], dt.float32, tag="p1")
                    p2 = psum.tile([128, CHUNK * W], dt.float32, tag="p2")
                    p3 = psum3.tile([128, CHUNK * W], dt.float32, tag="p3")
                    for ws, pp in ((w1s, p1), (w2s, p2)):
                        for mi in range(7):
                            if mi < 6:
                                ky, bsel = mi % 3, mi // 3
                                s0 = r0 + ky
                                rhs = cur[bsel][0:128, s0:s0 + nr, 2:2 + W]
                            else:
                                rhs = cur[2][0:128, 1 + r0:1 + r0 + nr,
                                             2:2 + W]
                            nc.tensor.matmul(
                                pp[:, :N],
                                ws[:, mi * 128:(mi + 1) * 128],
                                rhs, start=(mi == 0), stop=(mi == 6))

                    rz = work.tile([128, CHUNK * W], dt.bfloat16, tag="rz")
                    nc.scalar.activation(rz[:, :N], p1[:, :N], AF.Sigmoid)
                    q = work.tile([128, CHUNK * W], dt.float32, tag="q")
                    nc.vector.tensor_mul(q[0:64, :N], rz[0:64, :N],
                                         p2[0:64, :N])
                    nc.vector.tensor_add(p3[0:64, :N], q[0:64, :N],
                                         p2[64:128, :N])
                    n_t = work.tile([128, CHUNK * W], dt.bfloat16, tag="n_t")
                    nc.scalar.activation(n_t[64:128, :N], p3[0:64, :N],
                                         AF.Tanh)
                    d_t = work.tile([128, CHUNK * W], dt.bfloat16, tag="d_t")
                    nc.vector.tensor_sub(d_t[64:128, :N],
                                         cur[0][64:128, 1 + r0:1 + r0 + nr,
                                                2:2 + W],
                                         n_t[64:128, :N])
                    e_t = work.tile([128, CHUNK * W], dt.bfloat16, tag="e_t")
                    nc.vector.tensor_mul(e_t[64:128, :N], rz[64:128, :N],
                                         d_t[64:128, :N])
                    nc.vector.tensor_add(An[64:128, 1 + r0:1 + r0 + nr,
                                            2:2 + W],
                                         n_t[64:128, :N], e_t[64:128, :N])
                    if t < T:
                        f0, f1 = (1 + r0) * PC, (1 + r0 + nr) * PC
                        nc.sync.dma_start(An_f[0:64, f0 + 1:f1 + 1],
                                          An_f[64:128, f0:f1])
                        nc.sync.dma_start(Bn_f[0:64, f0 - 1:f1 - 1],
                                          An_f[64:128, f0:f1])
                # owned rows 1..32 flat, one DMA per step
                nc.sync.dma_start(out[t - 1], An_f[64:128, PC:PC + OUTSZ])

    nc.compile()
    return nc


def _pack_weights(Wi, Wh):
    """Build the two [128, 7*128] lhsT weight tables for one core."""
    mms = []
    for bsel in range(2):
        for ky in range(3):
            if bsel == 0:   # A: h@dx=-1 | h@dx=0
                mms.append([(0, 64, 'h', ky, 0), (64, 64, 'h', ky, 1)])
            else:           # B: h@dx=+1 | x@dx=-1 | x@dx=0
                mms.append([(0, 64, 'h', ky, 2), (64, 32, 'x', ky, 0),
                            (96, 32, 'x', ky, 1)])
    mms.append([(0, 32, 'x', 0, 2), (32, 32, 'x', 1, 2),
                (64, 32, 'x', 2, 2)])
    w1 = np.zeros((7, 128, 128), np.float32)
    w2 = np.zeros((7, 128, 128), np.float32)
    for i, atoms in enumerate(mms):
        for (p0, n, kind, ky, kx) in atoms:
            for c in range(n):
                p = p0 + c
                if kind == 'h':
                    w1[i, p, 0:64] = Wh[0:64, c, ky, kx]      # r gate
                    w1[i, p, 64:128] = Wh[64:128, c, ky, kx]  # z gate
                    w2[i, p, 0:64] = Wh[128:192, c, ky, kx]   # h_n
                else:
                    w1[i, p, 0:64] = Wi[0:64, c, ky, kx]
                    w1[i, p, 64:128] = Wi[64:128, c, ky, kx]
                    w2[i, p, 64:128] = Wi[128:192, c, ky, kx]  # i_n
    w1 = np.ascontiguousarray(w1.transpose(1, 0, 2).reshape(128, 7 * 128))
    w2 = np.ascontiguousarray(w2.transpose(1, 0, 2).reshape(128, 7 * 128))
    return w1, w2


def _prep_inputs(x, W_i, W_h):
    import ml_dtypes

    bf16 = ml_dtypes.bfloat16
    in_maps = []
    for c in range(NCORES):
        b, half = divmod(c, 2)
        xs = x[b]                      # [T, CIN, H, W]
        Wi, Wh = W_i, W_h
        if half == 1:
            xs = xs[:, :, ::-1, :]
            Wi = W_i[:, :, ::-1, :]
            Wh = W_h[:, :, ::-1, :]
        # pre-padded canonical layout: x[r, c] at row 1+r, col 2+c
        xp = np.zeros((T, CIN, PR, PC), np.float32)
        xp[:, :, 1:1 + XR, 2:2 + W] = xs[:, :, :XR, :]
        xp = np.ascontiguousarray(xp.reshape(T, CIN, FLAT)).astype(bf16)
        w1, w2 = _pack_weights(Wi, Wh)
        in_maps.append({"xin": xp, "w1": w1.astype(bf16),
                        "w2": w2.astype(bf16)})
    return in_maps


def kernel(x, W_i, W_h):
    from concourse.bass_utils import run_bass_kernel_spmd

    x = np.asarray(x, dtype=np.float32)
    W_i = np.asarray(W_i, dtype=np.float32)
    W_h = np.asarray(W_h, dtype=np.float32)

    if "nc" not in _CACHE:
        _CACHE["nc"] = _build()
    nc = _CACHE["nc"]

    in_maps = _prep_inputs(x, W_i, W_h)
    trace = bool(os.environ.get("BASS_TRACE"))
    res = run_bass_kernel_spmd(nc, in_maps, list(range(NCORES)), trace=trace)
    KERNEL_STATS["exec_time_ns"] = res.exec_time_ns
    KERNEL_STATS["trace"] = res.instructions_and_trace

    y = np.empty((B, T, HID, H, W), np.float32)
    for c in range(NCORES):
        b, half = divmod(c, 2)
        oc = np.asarray(res.results[c]["out"]).astype(np.float32)
        oc = oc.reshape(T, HID, OWN, PC)[:, :, :, 2:2 + W]
        if half == 0:
            y[b, :, :, 0:OWN, :] = oc
        else:
            y[b, :, :, OWN:H, :] = oc[:, :, ::-1, :]
    return y


# revision 7
# speedup vs baseline: 1.2231x; 1.2231x over previous
"""ConvGRU Trainium2 kernel, v3 (v2 + flat-contiguous DMA).

Same compute structure as v2 (14 K-packed matmuls/chunk via shift-baked
A/B/C rhs buffers; bf16 GRU chain; bf16 h state inside the buffers), but all
hot DMAs are single-run-per-partition flat copies:

- rhs buffers are [128, PR*PC] with PC=70. A shift by (dy, dx) is a flat
  element offset of -(70*dy + dx), so shifted replicas/loads are contiguous
  flat copies whose row-wrap elements land in always-zero pad columns.
- x arrives from DRAM pre-padded per core as xp[T, CIN, PR*PC] (canonical
  position s=1+r, u=2+c); the 5 per-step loads are flat shifted copies.
- h replicas (dx=-1 into A lower, dx=+1 into B lower) are flat +-1 copies
  of the canonical h@dx=0 chunk slice.
- output is one flat per-step DMA of the owned 32 rows (with pad cols),
  descrambled and upcast to f32 on the host.
"""

import os
import sys

sys.path.insert(0, "/opt/trn_rl_repo")

import numpy as np

T, CIN, HID, H, W = 16, 32, 64, 64, 64
B = 4
NCORES = 8
OWN = 32           # owned H rows per core
XR = 48            # x slice rows fed to each core
PR = 50            # padded rows in rhs buffers
PC = 70            # padded cols (even: keeps DVE rows 4B-aligned)
CHUNK = 8          # output rows per chunk (8*64 = 512 = one PSUM bank)
FLAT = PR * PC
OUTSZ = OWN * PC   # owned-rows flat slice per step (rows 1..32)

_CACHE = {}
KERNEL_STATS = {}


def _n_rows(t):
    return OWN + (T - t)


def _build():
    import concourse.bacc as bacc
    import concourse.mybir as mybir
    from concourse import tile

    dt = mybir.dt
    AF = mybir.ActivationFunctionType

    nc = bacc.Bacc("TRN2", target_bir_lowering=False, debug=False,
                   num_devices=NCORES)
    xin = nc.dram_tensor("xin", [T, CIN, FLAT], dt.bfloat16,
                         kind="ExternalInput")
    w1 = nc.dram_tensor("w1", [128, 7 * 128], dt.bfloat16,
                        kind="ExternalInput")
    w2 = nc.dram_tensor("w2", [128, 7 * 128], dt.bfloat16,
                        kind="ExternalInput")
    out = nc.dram_tensor("out", [T, HID, OUTSZ], dt.bfloat16,
                         kind="ExternalOutput")

    # x data occupies flat [70, 3430) (rows 1..48, canonical). A target with
    # baked shift (dy, dx) holds it at flat offset d = -(70*dy + dx).
    xspan = (PC, (1 + XR) * PC)

    with tile.TileContext(nc) as tc:
        with tc.tile_pool(name="const", bufs=1) as const, \
             tc.tile_pool(name="state", bufs=1) as state, \
             tc.tile_pool(name="work", bufs=3) as work, \
             tc.tile_pool(name="psum", bufs=2, space="PSUM") as psum:

            w1s = const.tile([128, 7 * 128], dt.bfloat16, tag="w1s")
            w2s = const.tile([128, 7 * 128], dt.bfloat16, tag="w2s")
            nc.sync.dma_start(w1s[:], w1[:])
            nc.sync.dma_start(w2s[:], w2[:])

            bufs, bufs3 = [], []
            for s in range(2):
                trio, trio3 = [], []
                for nm in ("A", "B", "C"):
                    tl = state.tile([128, FLAT], dt.bfloat16,
                                    tag=f"{nm}{s}", name=f"{nm}{s}")
                    nc.gpsimd.memset(tl[:], 0.0)
                    trio.append(tl)
                    trio3.append(tl.rearrange("p (r c) -> p r c", c=PC))
                bufs.append(trio)
                bufs3.append(trio3)

            def load_x(t, ds):
                # (partition block, dy, dx) -> flat shifted copy of xin[t]
                _, Bb, Cb = bufs[ds]
                lo, hi = xspan
                for tl, p0, dy, dx in ((Bb, 64, 0, -1), (Bb, 96, 0, 0),
                                       (Cb, 0, -1, 1), (Cb, 32, 0, 1),
                                       (Cb, 64, 1, 1)):
                    d = -(PC * dy + dx)
                    s0, s1 = max(0, lo + d), min(FLAT, hi + d)  # dst range
                    nc.sync.dma_start(tl[p0:p0 + 32, s0:s1],
                                      xin[t, :, s0 - d:s1 - d])

            load_x(0, 0)

            for t in range(1, T + 1):
                nt = _n_rows(t)
                cur = bufs3[(t - 1) % 2]
                An_f, Bn_f, _ = bufs[t % 2]
                An = bufs3[t % 2][0]

                if t < T:
                    load_x(t, t % 2)

                nchunks = (nt + CHUNK - 1) // CHUNK
                for ci in range(nchunks):
                    r0 = ci * CHUNK
                    nr = min(CHUNK, nt - r0)
                    N = nr * W
                    p1 = psum.tile([128, CHUNK * W], dt.float32, tag="p1")
                    p2 = psum.tile([128, CHUNK * W], dt.float32, tag="p2")
                    p3 = psum.tile([128, CHUNK * W], dt.float32, tag="p3")
                    for ws, pp in ((w1s, p1), (w2s, p2)):
                        for mi in range(7):
                            if mi < 6:
                                ky, bsel = mi % 3, mi // 3
                                s0 = r0 + ky
                                rhs = cur[bsel][0:128, s0:s0 + nr, 2:2 + W]
                            else:
                                rhs = cur[2][0:128, 1 + r0:1 + r0 + nr,
                                             2:2 + W]
                            nc.tensor.matmul(
                                pp[:, :N],
                                ws[:, mi * 128:(mi + 1) * 128],
                                rhs, start=(mi == 0), stop=(mi == 6))

                    rz = work.tile([128, CHUNK * W], dt.bfloat16, tag="rz")
                    nc.scalar.activation(rz[:, :N], p1[:, :N], AF.Sigmoid)
                    q = work.tile([128, CHUNK * W], dt.float32, tag="q")
                    nc.vector.tensor_mul(q[0:64, :N], rz[0:64, :N],
                                         p2[0:64, :N])
                    nc.vector.tensor_add(p3[0:64, :N], q[0:64, :N],
                                         p2[64:128, :N])
                    n_t = work.tile([128, CHUNK * W], dt.bfloat16, tag="n_t")
                    nc.scalar.activation(n_t[64:128, :N], p3[0:64, :N],
                                         AF.Tanh)
                    d_t = work.tile([128, CHUNK * W], dt.bfloat16, tag="d_t")
                    nc.vector.tensor_sub(d_t[64:128, :N],
                                         cur[0][64:128, 1 + r0:1 + r0 + nr,
                                                2:2 + W],
                                         n_t[64:128, :N])
                    e_t = work.tile([128, CHUNK * W], dt.bfloat16, tag="e_t")
                    nc.vector.tensor_mul(e_t[64:128, :N], rz[64:128, :N],
                                         d_t[64:128, :N])
                    nc.vector.tensor_add(An[64:128, 1 + r0:1 + r0 + nr,
                                            2:2 + W],
                                         n_t[64:128, :N], e_t[64:128, :N])
                    if t < T:
                        f0, f1 = (1 + r0) * PC, (1 + r0 + nr) * PC
                        nc.sync.dma_start(An_f[0:64, f0 + 1:f1 + 1],
                                          An_f[64:128, f0:f1])
                        nc.sync.dma_start(Bn_f[0:64, f0 - 1:f1 - 1],
                                          An_f[64:128, f0:f1])
                # owned rows 1..32 flat, one DMA per step
                nc.sync.dma_start(out[t - 1], An_f[64:128, PC:PC + OUTSZ])

    nc.compile()
    return nc


def _pack_weights(Wi, Wh):
    """Build the two [128, 7*128] lhsT weight tables for one core."""
    mms = []
    for bsel in range(2):
        for ky in range(3):
            if bsel == 0:   # A: h@dx=-1 | h@dx=0
                mms.append([(0, 64, 'h', ky, 0), (64, 64, 'h', ky, 1)])
            else:           # B: h@dx=+1 | x@dx=-1 | x@dx=0
                mms.append([(0, 64, 'h', ky, 2), (64, 32, 'x', ky, 0),
                            (96, 32, 'x', ky, 1)])
    mms.append([(0, 32, 'x', 0, 2), (32, 32, 'x', 1, 2),
                (64, 32, 'x', 2, 2)])
    w1 = np.zeros((7, 128, 128), np.float32)
    w2 = np.zeros((7, 128, 128), np.float32)
    for i, atoms in enumerate(mms):
        for (p0, n, kind, ky, kx) in atoms:
            for c in range(n):
                p = p0 + c
                if kind == 'h':
                    w1[i, p, 0:64] = Wh[0:64, c, ky, kx]      # r gate
                    w1[i, p, 64:128] = Wh[64:128, c, ky, kx]  # z gate
                    w2[i, p, 0:64] = Wh[128:192, c, ky, kx]   # h_n
                else:
                    w1[i, p, 0:64] = Wi[0:64, c, ky, kx]
                    w1[i, p, 64:128] = Wi[64:128, c, ky, kx]
                    w2[i, p, 64:128] = Wi[128:192, c, ky, kx]  # i_n
    w1 = np.ascontiguousarray(w1.transpose(1, 0, 2).reshape(128, 7 * 128))
    w2 = np.ascontiguousarray(w2.transpose(1, 0, 2).reshape(128, 7 * 128))
    return w1, w2


def _prep_inputs(x, W_i, W_h):
    import ml_dtypes

    bf16 = ml_dtypes.bfloat16
    in_maps = []
    for c in range(NCORES):
        b, half = divmod(c, 2)
        xs = x[b]                      # [T, CIN, H, W]
        Wi, Wh = W_i, W_h
        if half == 1:
            xs = xs[:, :, ::-1, :]
            Wi = W_i[:, :, ::-1, :]
            Wh = W_h[:, :, ::-1, :]
        # pre-padded canonical layout: x[r, c] at row 1+r, col 2+c
        xp = np.zeros((T, CIN, PR, PC), np.float32)
        xp[:, :, 1:1 + XR, 2:2 + W] = xs[:, :, :XR, :]
        xp = np.ascontiguousarray(xp.reshape(T, CIN, FLAT)).astype(bf16)
        w1, w2 = _pack_weights(Wi, Wh)
        in_maps.append({"xin": xp, "w1": w1.astype(bf16),
                        "w2": w2.astype(bf16)})
    return in_maps


def kernel(x, W_i, W_h):
    from concourse.bass_utils import run_bass_kernel_spmd

    x = np.asarray(x, dtype=np.float32)
    W_i = np.asarray(W_i, dtype=np.float32)
    W_h = np.asarray(W_h, dtype=np.float32)

    if "nc" not in _CACHE:
        _CACHE["nc"] = _build()
    nc = _CACHE["nc"]

    in_maps = _prep_inputs(x, W_i, W_h)
    trace = bool(os.environ.get("BASS_TRACE"))
    res = run_bass_kernel_spmd(nc, in_maps, list(range(NCORES)), trace=trace)
    KERNEL_STATS["exec_time_ns"] = res.exec_time_ns
    KERNEL_STATS["trace"] = res.instructions_and_trace

    y = np.empty((B, T, HID, H, W), np.float32)
    for c in range(NCORES):
        b, half = divmod(c, 2)
        oc = np.asarray(res.results[c]["out"]).astype(np.float32)
        oc = oc.reshape(T, HID, OWN, PC)[:, :, :, 2:2 + W]
        if half == 0:
            y[b, :, :, 0:OWN, :] = oc
        else:
            y[b, :, :, OWN:H, :] = oc[:, :, ::-1, :]
    return y


# revision 8
# speedup vs baseline: 1.2233x; 1.0002x over previous
"""ConvGRU Trainium2 kernel, v3 (v2 + flat-contiguous DMA).

Same compute structure as v2 (14 K-packed matmuls/chunk via shift-baked
A/B/C rhs buffers; bf16 GRU chain; bf16 h state inside the buffers), but all
hot DMAs are single-run-per-partition flat copies:

- rhs buffers are [128, PR*PC] with PC=70. A shift by (dy, dx) is a flat
  element offset of -(70*dy + dx), so shifted replicas/loads are contiguous
  flat copies whose row-wrap elements land in always-zero pad columns.
- x arrives from DRAM pre-padded per core as xp[T, CIN, PR*PC] (canonical
  position s=1+r, u=2+c); the 5 per-step loads are flat shifted copies.
- h replicas (dx=-1 into A lower, dx=+1 into B lower) are flat +-1 copies
  of the canonical h@dx=0 chunk slice.
- output is one flat per-step DMA of the owned 32 rows (with pad cols),
  descrambled and upcast to f32 on the host.
"""

import os
import sys

sys.path.insert(0, "/opt/trn_rl_repo")

import numpy as np

T, CIN, HID, H, W = 16, 32, 64, 64, 64
B = 4
NCORES = 8
OWN = 32           # owned H rows per core
XR = 48            # x slice rows fed to each core
PR = 50            # padded rows in rhs buffers
PC = 70            # padded cols (even: keeps DVE rows 4B-aligned)
CHUNK = 8          # output rows per chunk (8*64 = 512 = one PSUM bank)
FLAT = PR * PC
OUTSZ = OWN * PC   # owned-rows flat slice per step (rows 1..32)

_CACHE = {}
KERNEL_STATS = {}


def _n_rows(t):
    return OWN + (T - t)


def _build():
    import concourse.bacc as bacc
    import concourse.mybir as mybir
    from concourse import tile

    dt = mybir.dt
    AF = mybir.ActivationFunctionType

    nc = bacc.Bacc("TRN2", target_bir_lowering=False, debug=False,
                   num_devices=NCORES)
    xin = nc.dram_tensor("xin", [T, CIN, FLAT], dt.bfloat16,
                         kind="ExternalInput")
    w1 = nc.dram_tensor("w1", [128, 7 * 128], dt.bfloat16,
                        kind="ExternalInput")
    w2 = nc.dram_tensor("w2", [128, 7 * 128], dt.bfloat16,
                        kind="ExternalInput")
    out = nc.dram_tensor("out", [T, HID, OUTSZ], dt.bfloat16,
                         kind="ExternalOutput")

    # x data occupies flat [70, 3430) (rows 1..48, canonical). A target with
    # baked shift (dy, dx) holds it at flat offset d = -(70*dy + dx).
    xspan = (PC, (1 + XR) * PC)

    with tile.TileContext(nc) as tc:
        with tc.tile_pool(name="const", bufs=1) as const, \
             tc.tile_pool(name="state", bufs=1) as state, \
             tc.tile_pool(name="work", bufs=3) as work, \
             tc.tile_pool(name="psum", bufs=3, space="PSUM") as psum, \
             tc.tile_pool(name="psum3", bufs=2, space="PSUM") as psum3:

            w1s = const.tile([128, 7 * 128], dt.bfloat16, tag="w1s")
            w2s = const.tile([128, 7 * 128], dt.bfloat16, tag="w2s")
            # per-block loads so MM 0 only waits on its own 128 columns
            for mi in range(7):
                sl = slice(mi * 128, (mi + 1) * 128)
                nc.sync.dma_start(w1s[:, sl], w1[:, sl])
                nc.sync.dma_start(w2s[:, sl], w2[:, sl])

            # zero-fill spread across engines so startup is not serialized
            # on one engine (DVE memset on [128, 3500] bf16 runs ~1 us)
            bufs, bufs3 = [], []
            zeng = [lambda ap: nc.vector.memset(ap, 0.0),
                    lambda ap: nc.scalar.memzero(ap),
                    lambda ap: nc.gpsimd.memset(ap, 0.0)]
            for s in range(3):
                trio, trio3 = [], []
                for k, nm in enumerate(("B", "C", "A")):
                    tl = state.tile([128, FLAT], dt.bfloat16,
                                    tag=f"{nm}{s}", name=f"{nm}{s}")
                    if s == 0:
                        zeng[k](tl[0:64, :])
                        zeng[(k + 1) % 3](tl[64:128, :])
                    else:
                        zeng[k](tl[:])
                    trio.append(tl)
                    trio3.append(tl.rearrange("p (r c) -> p r c", c=PC))
                # emitted B, C, A; store as A, B, C
                bufs.append([trio[2], trio[0], trio[1]])
                bufs3.append([trio3[2], trio3[0], trio3[1]])

            def load_x(t, ds, split=None):
                # (partition block, dy, dx) -> flat shifted copy of xin[t].
                # split: emit rows < split first so early chunks can start.
                _, Bb, Cb = bufs[ds]
                lo, hi = xspan
                tgts = ((Bb, 64, 0, -1), (Bb, 96, 0, 0),
                        (Cb, 0, -1, 1), (Cb, 32, 0, 1), (Cb, 64, 1, 1))
                pieces = [(lo, hi)] if split is None else \
                    [(lo, lo + split * PC), (lo + split * PC, hi)]
                for plo, phi in pieces:
                    for tl, p0, dy, dx in tgts:
                        d = -(PC * dy + dx)
                        s0, s1 = max(0, plo + d), min(FLAT, phi + d)
                        nc.sync.dma_start(tl[p0:p0 + 32, s0:s1],
                                          xin[t, :, s0 - d:s1 - d])

            load_x(0, 0, split=12)

            for t in range(1, T + 1):
                nt = _n_rows(t)
                cur = bufs3[(t - 1) % 3]
                An_f, Bn_f, _ = bufs[t % 3]
                An = bufs3[t % 3][0]

                if t < T:
                    load_x(t, t % 3)

                nchunks = (nt + CHUNK - 1) // CHUNK
                for ci in range(nchunks):
                    r0 = ci * CHUNK
                    nr = min(CHUNK, nt - r0)
                    N = nr * W
                    p1 = psum.tile([128, CHUNK * W], dt.float32, tag="p1")
                    p2 = psum.tile([128, CHUNK * W], dt.float32, tag="p2")
                    p3 = psum3.tile([128, CHUNK * W], dt.float32, tag="p3")
                    # ky=2 views (A2=2, B2=5) read the deepest rows of the
                    # previous step's h and can stall at step boundaries;
                    # emit them last so 5 of 7 MMs issue unconditionally.
                    mm_order = (0, 1, 3, 4, 6, 2, 5)
                    for ws, pp in ((w1s, p1), (w2s, p2)):
                        for oi, mi in enumerate(mm_order):
                            if mi < 6:
                                ky, bsel = mi % 3, mi // 3
                                s0 = r0 + ky
                                rhs = cur[bsel][0:128, s0:s0 + nr, 2:2 + W]
                            else:
                                rhs = cur[2][0:128, 1 + r0:1 + r0 + nr,
                                             2:2 + W]
                            nc.tensor.matmul(
                                pp[:, :N],
                                ws[:, mi * 128:(mi + 1) * 128],
                                rhs, start=(oi == 0), stop=(oi == 6))

                    rz = work.tile([128, CHUNK * W], dt.bfloat16, tag="rz")
                    nc.scalar.activation(rz[:, :N], p1[:, :N], AF.Sigmoid)
                    q = work.tile([128, CHUNK * W], dt.float32, tag="q")
                    nc.vector.tensor_mul(q[0:64, :N], rz[0:64, :N],
                                         p2[0:64, :N])
                    nc.vector.tensor_add(p3[0:64, :N], q[0:64, :N],
                                         p2[64:128, :N])
                    n_t = work.tile([128, CHUNK * W], dt.bfloat16, tag="n_t")
                    nc.scalar.activation(n_t[64:128, :N], p3[0:64, :N],
                                         AF.Tanh)
                    d_t = work.tile([128, CHUNK * W], dt.bfloat16, tag="d_t")
                    nc.vector.tensor_sub(d_t[64:128, :N],
                                         cur[0][64:128, 1 + r0:1 + r0 + nr,
                                                2:2 + W],
                                         n_t[64:128, :N])
                    e_t = work.tile([128, CHUNK * W], dt.bfloat16, tag="e_t")
                    nc.vector.tensor_mul(e_t[64:128, :N], rz[64:128, :N],
                                         d_t[64:128, :N])
                    nc.vector.tensor_add(An[64:128, 1 + r0:1 + r0 + nr,
                                            2:2 + W],
                                         n_t[64:128, :N], e_t[64:128, :N])
                    if t < T:
                        f0, f1 = (1 + r0) * PC, (1 + r0 + nr) * PC
                        nc.sync.dma_start(An_f[0:64, f0 + 1:f1 + 1],
                                          An_f[64:128, f0:f1])
                        nc.sync.dma_start(Bn_f[0:64, f0 - 1:f1 - 1],
                                          An_f[64:128, f0:f1])
                # owned rows 1..32 flat, one DMA per step
                nc.sync.dma_start(out[t - 1], An_f[64:128, PC:PC + OUTSZ])

    nc.compile()
    return nc


def _pack_weights(Wi, Wh):
    """Build the two [128, 7*128] lhsT weight tables for one core."""
    mms = []
    for bsel in range(2):
        for ky in range(3):
            if bsel == 0:   # A: h@dx=-1 | h@dx=0
                mms.append([(0, 64, 'h', ky, 0), (64, 64, 'h', ky, 1)])
            else:           # B: h@dx=+1 | x@dx=-1 | x@dx=0
                mms.append([(0, 64, 'h', ky, 2), (64, 32, 'x', ky, 0),
                            (96, 32, 'x', ky, 1)])
    mms.append([(0, 32, 'x', 0, 2), (32, 32, 'x', 1, 2),
                (64, 32, 'x', 2, 2)])
    w1 = np.zeros((7, 128, 128), np.float32)
    w2 = np.zeros((7, 128, 128), np.float32)
    for i, atoms in enumerate(mms):
        for (p0, n, kind, ky, kx) in atoms:
            for c in range(n):
                p = p0 + c
                if kind == 'h':
                    w1[i, p, 0:64] = Wh[0:64, c, ky, kx]      # r gate
                    w1[i, p, 64:128] = Wh[64:128, c, ky, kx]  # z gate
                    w2[i, p, 0:64] = Wh[128:192, c, ky, kx]   # h_n
                else:
                    w1[i, p, 0:64] = Wi[0:64, c, ky, kx]
                    w1[i, p, 64:128] = Wi[64:128, c, ky, kx]
                    w2[i, p, 64:128] = Wi[128:192, c, ky, kx]  # i_n
    w1 = np.ascontiguousarray(w1.transpose(1, 0, 2).reshape(128, 7 * 128))
    w2 = np.ascontiguousarray(w2.transpose(1, 0, 2).reshape(128, 7 * 128))
    return w1, w2


def _prep_inputs(x, W_i, W_h):
    import ml_dtypes

    bf16 = ml_dtypes.bfloat16
    in_maps = []
    for c in range(NCORES):
        b, half = divmod(c, 2)
        xs = x[b]                      # [T, CIN, H, W]
        Wi, Wh = W_i, W_h
        if half == 1:
            xs = xs[:, :, ::-1, :]
            Wi = W_i[:, :, ::-1, :]
            Wh = W_h[:, :, ::-1, :]
        # pre-padded canonical layout: x[r, c] at row 1+r, col 2+c
        xp = np.zeros((T, CIN, PR, PC), np.float32)
        xp[:, :, 1:1 + XR, 2:2 + W] = xs[:, :, :XR, :]
        xp = np.ascontiguousarray(xp.reshape(T, CIN, FLAT)).astype(bf16)
        w1, w2 = _pack_weights(Wi, Wh)
        in_maps.append({"xin": xp, "w1": w1.astype(bf16),
                        "w2": w2.astype(bf16)})
    return in_maps


def kernel(x, W_i, W_h):
    from concourse.bass_utils import run_bass_kernel_spmd

    x = np.asarray(x, dtype=np.float32)
    W_i = np.asarray(W_i, dtype=np.float32)
    W_h = np.asarray(W_h, dtype=np.float32)

    if "nc" not in _CACHE:
        _CACHE["nc"] = _build()
    nc = _CACHE["nc"]

    in_maps = _prep_inputs(x, W_i, W_h)
    trace = bool(os.environ.get("BASS_TRACE"))
    res = run_bass_kernel_spmd(nc, in_maps, list(range(NCORES)), trace=trace)
    KERNEL_STATS["exec_time_ns"] = res.exec_time_ns
    KERNEL_STATS["trace"] = res.instructions_and_trace

    y = np.empty((B, T, HID, H, W), np.float32)
    for c in range(NCORES):
        b, half = divmod(c, 2)
        oc = np.asarray(res.results[c]["out"]).astype(np.float32)
        oc = oc.reshape(T, HID, OWN, PC)[:, :, :, 2:2 + W]
        if half == 0:
            y[b, :, :, 0:OWN, :] = oc
        else:
            y[b, :, :, OWN:H, :] = oc[:, :, ::-1, :]
    return y
